# revision 16
# baseline (speedup 1.0000x reference)
"""Trainium2 Bass kernel for nn_Attention_4449586119407.

GQA attention layer (B=2, L=2048, D=2048, 32 Q heads / 8 KV heads, RoPE,
causal) sharded over 8 NeuronCores: data-parallel over batch (2) x
tensor-parallel over heads (4 groups of 8 Q heads / 2 KV heads).
wq/wk/wv column-sharded, wo row-sharded; the wo partial sums are reduced
on the host.

Device-side layout (per core):
  - All matmuls contract over the SBUF partition dim.  x is pre-transposed
    on the host (xT [D, L]) so QKV projections produce channel-major
    qT/kT [c, l] directly.
  - RoPE pairs are made partition-contiguous by permuting wq/wk rows on the
    host (per head: even rows then odd rows -> [te(32), to(32)] blocks).
    The rotation partner (partition swap te<->to) is produced with 4 small
    SBUF->SBUF DMAs; the rotation itself is 3 full-width DVE ops using
    host-precomputed cos / signed-sin maps.
  - Scores are computed transposed (S[j, i] = k . q) so the softmaxed tile
    can feed the P@V matmul directly as the stationary operand.  Softmax
    skips the max-subtraction (logits are O(5) here) and gets the
    denominator for free as a 65th "ones" column of V.
  - Causality is structural: only lower-triangle j-blocks are computed and
    the 128x128 diagonal blocks get the (transposed) mask block added in
    PSUM before the exp.
"""

import numpy as np

B, L, D = 2, 2048, 2048
NH, NKV, HD = 32, 8, 64
SCALE = HD ** -0.5
NCORES = 8
F32 = np.float32
SHUF_MASK = [(i + 16) % 32 for i in range(32)]

_CACHE = {}


def _build_nc(debug=False, weave=False, tweak=False, den_v=False,
              tweak2=False, tweak3=False, tweak4=False, tweak5=False,
              tweak6=False, tweak7=False, tweak8=False,
              warm2=0, trf16=False, dennorm=False, vtg=False, ysbg=False):
    from contextlib import ExitStack

    import concourse.tile as tile
    from concourse import bacc, mybir

    f32 = mybir.dt.float32
    f32r = mybir.dt.float32r
    f16 = mybir.dt.float16
    AF = mybir.ActivationFunctionType
    ALU = mybir.AluOpType

    nc = bacc.Bacc("TRN2", target_bir_lowering=False, debug=False,
                   num_devices=NCORES)

    xT = nc.dram_tensor("xT", [D, L], f16, kind="ExternalInput").ap()
    wq_sb_d = nc.dram_tensor("wq_sb", [128, 8192], f16, kind="ExternalInput").ap()
    wk_sb_d = nc.dram_tensor("wk_sb", [128, 2048], f16, kind="ExternalInput").ap()
    wv_sb_d = nc.dram_tensor("wv_sb", [128, 2048], f16, kind="ExternalInput").ap()
    wo_sb_d = nc.dram_tensor("wo_sb", [128, 8192], f16, kind="ExternalInput").ap()
    mapdt = f16 if tweak7 else f32
    cosm_d = nc.dram_tensor("cosm", [128, 2048], mapdt, kind="ExternalInput").ap()
    sinm2_d = nc.dram_tensor("sinm2", [128, 2048], mapdt, kind="ExternalInput").ap()
    maskT_d = nc.dram_tensor("maskT", [128, 128], f32, kind="ExternalInput").ap()
    identdt = f16 if trf16 else f32
    ident_d = nc.dram_tensor("ident", [128, 128], identdt, kind="ExternalInput").ap()
    y_d = nc.dram_tensor("y", [L, D], f16 if tweak6 else f32,
                         kind="ExternalOutput").ap()
    if debug:
        dbg = {nm: nc.dram_tensor(f"dbg_{nm}", sh, dt, kind="ExternalOutput").ap()
               for nm, sh, dt in [
                   ("q00", [128, 512], f16), ("kz00", [128, 2048], f16),
                   ("vext0", [128, 2048], f16), ("p000", [128, 1024], f16),
                   ("attT00", [128, 512], f16), ("att0", [128, 512], f32), ("rcpb0", [64, 512], f32)]}

    with tile.TileContext(nc) as tc:
        with ExitStack() as ctx:
            singles = ctx.enter_context(tc.tile_pool(name="singles", bufs=1))
            xt_p = ctx.enter_context(tc.tile_pool(name="xt", bufs=24))
            maps_p = ctx.enter_context(tc.tile_pool(name="maps", bufs=4))
            qrot_p = ctx.enter_context(tc.tile_pool(name="qrot", bufs=8))
            ptile_p = ctx.enter_context(tc.tile_pool(name="pt", bufs=4))
            tmp_p = ctx.enter_context(tc.tile_pool(name="tmp", bufs=3))
            attT_p = ctx.enter_context(tc.tile_pool(name="attT", bufs=13))
            rcpb_p = ctx.enter_context(tc.tile_pool(name="rcpb", bufs=2))
            ysb_p = ctx.enter_context(tc.tile_pool(name="ysb",
                                                   bufs=8 if tweak8 else 3))
            pj_ps = ctx.enter_context(tc.tile_pool(name="pj", bufs=2, space="PSUM"))
            sc_ps = ctx.enter_context(tc.tile_pool(name="sc", bufs=2, space="PSUM"))
            at_ps = ctx.enter_context(tc.tile_pool(name="at", bufs=2, space="PSUM"))

            # ---- HAM warmup: keep PE busy while the first DMAs land ----
            if warm2:
                # 512-row warm matmuls sized to end as xt chunk 0 lands, with
                # the PE clock fully ramped (LOW 100ns -> MID 3us -> FULL)
                warm_sb = singles.tile([128, 512], f16)
                nc.vector.memset(warm_sb[:], 0.0)
                warm_ps = pj_ps.tile([64, 512], f32, tag="pj", name="warm_ps")
                for _ in range(warm2):
                    nc.tensor.matmul(warm_ps[:], warm_sb[:, 0:64], warm_sb[:],
                                     start=True, stop=True)
            else:
                warm_sb = singles.tile([128, 128], f16)
                nc.vector.memset(warm_sb[:], 0.0)
                warm_ps = pj_ps.tile([64, 64], f32, tag="pj", name="warm_ps")
                for _ in range(30 if tweak2 else 90):
                    nc.tensor.matmul(warm_ps[:], warm_sb[:, 0:64],
                                     warm_sb[:, 0:64], start=True, stop=True)

            # ---- resident constants (wq cb0 first; bulk after xt chunk 0) ----
            # cb0 split into 512-col chunks so the first q-chain stationaries
            # land sooner
            wq_sb = singles.tile([128, 8192], f16)
            for c in range(4):
                nc.sync.dma_start(wq_sb[:, c * 512:(c + 1) * 512],
                                  wq_sb_d[:, c * 512:(c + 1) * 512])
            wk_sb = singles.tile([128, 2048], f16)
            wv_sb = singles.tile([128, 2048], f16)
            mask_sb = singles.tile([128, 128], f32)
            ident_sb = singles.tile([128, 128], identdt)
            wo_sb = singles.tile([128, 8192], f16)

            def load_weights_bulk():
                for cb in range(1, 4):
                    nc.sync.dma_start(wq_sb[:, cb * 2048:(cb + 1) * 2048],
                                      wq_sb_d[:, cb * 2048:(cb + 1) * 2048])
                nc.sync.dma_start(wk_sb[:], wk_sb_d[:])
                nc.sync.dma_start(wv_sb[:], wv_sb_d[:])
                nc.sync.dma_start(mask_sb[:], maskT_d[:])
                nc.sync.dma_start(ident_sb[:], ident_d[:])
            # kz[kv][half]: rotated k for kv head, in partition half `half`,
            # other half zero -> K=128 scores matmuls with full partitions
            kz = [[singles.tile([128, 2048], f16, name=f"kz{kv}{hf}")
                   for hf in range(2)] for kv in range(2)]
            for kv in range(2):
                # only the pad half must be zero; the data half is written
                nc.vector.memset(kz[kv][0][64:128, :], 0.0)
                nc.vector.memset(kz[kv][1][0:64, :], 0.0)
            # per jb: [v(64) | ones(1) | pad(63)] -> full 128-col stationary;
            # with dennorm the pad becomes 64 ones-columns so the PV matmul
            # replicates the softmax denominator into psum partitions 64-127
            # (normalize then needs no partition broadcast)
            vext = [singles.tile([128, 2048], f16, name=f"vext{kv}")
                    for kv in range(2)]
            for kv in range(2):
                if dennorm:
                    v3d = vext[kv].rearrange("p (j c) -> p j c", c=128)
                    nc.vector.memset(v3d[:, :, 64:128], 1.0)
                else:
                    nc.vector.memset(vext[kv][:, 64::128], 1.0)

            def diag_off(jb, lc):
                """col offset of the diagonal 128-block inside chunk lc, or None"""
                od = 128 * jb - 512 * lc
                return od if 0 <= od < 512 else None

            q_tiles = {}

            def load_xt(lc):
                lsl = slice(lc * 512, (lc + 1) * 512)
                xt = []
                for db in range(16):
                    t = xt_p.tile([128, 512], f16, tag="xt", name=f"xt{db}")
                    nc.sync.dma_start(t[:], xT[db * 128:(db + 1) * 128, lsl])
                    xt.append(t)
                cos_t = maps_p.tile([128, 512], mapdt, tag="cos", name="cos_t")
                nc.sync.dma_start(cos_t[:], cosm_d[:, lsl])
                sin_t = maps_p.tile([128, 512], mapdt, tag="sin", name="sin_t")
                nc.sync.dma_start(sin_t[:], sinm2_d[:, lsl])
                return xt, cos_t, sin_t

            def do_proj(lc, loaded):
                lsl = slice(lc * 512, (lc + 1) * 512)
                xt, cos_t, sin_t = loaded

                def rope(ps, dest):
                    """rotate [128,512] psum block into dest (SBUF).

                    RoPE pairs are laid out [te(16) | to(16)] per 32-partition
                    quadrant (host-side row perm), so the rotation partner is
                    an intra-quadrant 16-block swap -> one DVE stream_shuffle.
                    """
                    t2 = tmp_p.tile([128, 512], f32, tag="t2", bufs=2, name="t2")
                    nc.vector.tensor_tensor(t2[:], ps[:], sin_t[:], op=ALU.mult)
                    qsw = tmp_p.tile([128, 512], f32, tag="qsw", name="qsw")
                    nc.vector.stream_shuffle(qsw[:], t2[:], mask=SHUF_MASK)
                    t3 = tmp_p.tile([128, 512], f32, tag="t3", bufs=2, name="t3")
                    nc.vector.tensor_tensor(t3[:], ps[:], cos_t[:], op=ALU.mult)
                    nc.vector.tensor_tensor(dest[:], t3[:], qsw[:], op=ALU.add)

                def q_unit(cb):
                    ps = pj_ps.tile([128, 512], f32, tag="pj", name="ps_q")
                    for db in range(16):
                        nc.tensor.matmul(
                            ps[:], wq_sb[:, (cb * 16 + db) * 128:(cb * 16 + db + 1) * 128],
                            xt[db][:], start=(db == 0), stop=(db == 15))
                    qt = qrot_p.tile([128, 512], f16, name="qt")
                    rope(ps, qt)
                    q_tiles[(cb, lc)] = qt

                def k_unit():
                    # k projection + RoPE + zero-padded scatter
                    ps = pj_ps.tile([128, 512], f32, tag="pj", name="ps_k")
                    for db in range(16):
                        nc.tensor.matmul(
                            ps[:], wk_sb[:, db * 128:(db + 1) * 128],
                            xt[db][:], start=(db == 0), stop=(db == 15))
                    kraw = tmp_p.tile([128, 512], f16, tag="kraw", bufs=2,
                                      name="kraw")
                    rope(ps, kraw)
                    for kv in range(2):
                        nc.sync.dma_start(kz[kv][0][0:64, lsl],
                                          kraw[kv * 64:kv * 64 + 64, :])
                        nc.sync.dma_start(kz[kv][1][64:128, lsl],
                                          kraw[kv * 64:kv * 64 + 64, :])

                def v_unit():
                    # v projection (transposed) then PE-transpose per block
                    vtdt = f16 if trf16 else f32
                    vt = tmp_p.tile([128, 512], vtdt, tag="vt", bufs=2,
                                    name="vt")
                    ps = pj_ps.tile([128, 512], f32, tag="pj", name="ps_v")
                    for db in range(16):
                        nc.tensor.matmul(
                            ps[:], wv_sb[:, db * 128:(db + 1) * 128],
                            xt[db][:], start=(db == 0), stop=(db == 15))
                    if vtg:
                        # GpSimd is idle and vext isn't needed until the next
                        # att phase; keep Scalar free for exp, DVE for rope
                        nc.gpsimd.tensor_copy(vt[:], ps[:])
                    else:
                        nc.scalar.copy(vt[:], ps[:])
                    for j in range(4):
                        jb = 4 * lc + j
                        ps = pj_ps.tile([128, 128], vtdt if trf16 else f32,
                                        tag="pj", name="ps_t")
                        nc.tensor.transpose(ps[:], vt[:, j * 128:(j + 1) * 128],
                                            ident_sb[:])
                        for kv in range(2):
                            nc.vector.tensor_copy(
                                vext[kv][:, jb * 128:jb * 128 + 64],
                                ps[:, kv * 64:kv * 64 + 64])

                # first att pair needs q-cb0 + kz + vext: emit those first
                return [lambda: q_unit(0), k_unit, v_unit,
                        lambda: q_unit(1), lambda: q_unit(2),
                        lambda: q_unit(3)]

            def do_att(lc, pairs=(0, 1, 2, 3), fill=()):
                # fill: closures (proj/outproj chains) woven between jb
                # iterations so the PE always has ready work while exp runs
                fill = list(fill)
                fi = 0
                acc = 0.0
                njb = 4 * lc + 4
                per = len(fill) / (len(pairs) * njb) if fill else 0.0
                for t in pairs:
                    heads = (2 * t, 2 * t + 1)
                    aps_ = [at_ps.tile([128, 512], f32, tag="at", name=f"at{e}")
                            for e in range(2)]
                    P = None
                    for jb in range(njb):
                        o = max(0, 128 * jb - 512 * lc)
                        S = sc_ps.tile([128, 1024], f32, tag="sc", name="S")
                        for e, h in enumerate(heads):
                            kt = kz[h // 4][h % 2]
                            nc.tensor.matmul(
                                S[:, e * 512 + o:(e + 1) * 512],
                                kt[:, jb * 128:(jb + 1) * 128],
                                q_tiles[(h // 2, lc)][:, o:512],
                                start=True, stop=True)
                        od = diag_off(jb, lc)
                        if od is not None and tweak3:
                            for e in range(2):
                                sl = slice(e * 512 + od, e * 512 + od + 128)
                                nc.vector.tensor_tensor(S[:, sl], S[:, sl],
                                                        mask_sb[:], op=ALU.add)
                        elif od is not None:
                            s3 = S.rearrange("p (e c) -> p e c", e=2)[:, :, od:od + 128]
                            m3 = mask_sb[:].unsqueeze(1).broadcast_to([128, 2, 128])
                            nc.vector.tensor_tensor(s3, s3, m3, op=ALU.add)
                        P = ptile_p.tile([128, 1024], f16, name="P")
                        if o == 0:
                            nc.scalar.activation(P[:], S[:], AF.Exp)
                        elif tweak3:
                            for e in range(2):
                                sl = slice(e * 512 + o, (e + 1) * 512)
                                nc.scalar.activation(P[:, sl], S[:, sl], AF.Exp)
                        else:
                            s3 = S.rearrange("p (e c) -> p e c", e=2)[:, :, o:512]
                            p3 = P.rearrange("p (e c) -> p e c", e=2)[:, :, o:512]
                            nc.scalar.activation(p3, s3, AF.Exp)
                        for e, h in enumerate(heads):
                            kv = h // 4
                            nc.tensor.matmul(
                                aps_[e][:, o:512],
                                vext[kv][:, jb * 128:jb * 128 + 128],
                                P[:, e * 512 + o:(e + 1) * 512],
                                start=(jb == 0), stop=(jb == njb - 1),
                                skip_group_check=True)
                        acc += per
                        while fi < len(fill) and fi < int(acc + 1e-9):
                            fill[fi]()
                            fi += 1
                    if debug and lc == 0 and t == 0:
                        nc.sync.dma_start(dbg["p000"][:], P[:])
                    attT = attT_p.tile([128, 512], f16, name="attT")
                    q_tiles[("attT", lc, t)] = attT
                    if debug and lc == 0 and t == 0:
                        dbga = tmp_p.tile([128, 512], f32, tag="dbga", bufs=1,
                                          name="dbga")
                        nc.vector.tensor_copy(dbga[:], aps_[0][:])
                        nc.sync.dma_start(dbg["att0"][:], dbga[:])
                    if dennorm:
                        # psum rows 64-127 hold the denominator replicated by
                        # the ones-columns of vext: normalize is reciprocal +
                        # one multiply, both DVE, no partition broadcast
                        for e in range(2):
                            rcp = rcpb_p.tile([64, 512], f32, tag="rcp",
                                              name="rcp")
                            nc.vector.reciprocal_approx_fast(
                                out=rcp[:], in_=aps_[e][64:128, :])
                            nc.vector.tensor_tensor(
                                attT[64 * e:64 * e + 64, :], aps_[e][0:64, :],
                                rcp[:], op=ALU.mult)
                        continue
                    if tweak5 and lc == 3 and t == 3:
                        # last pair gates the outproj(3) tail: normalize in
                        # i-halves, both heads' first half first, so lb0/1
                        # output chains start while the second half drains
                        for hh in range(2):
                            sl = slice(hh * 256, (hh + 1) * 256)
                            for e in range(2):
                                den = rcpb_p.tile([1, 256], f32, tag="denh",
                                                  bufs=2, name="denh")
                                nc.vector.tensor_copy(den[:],
                                                      aps_[e][64:65, sl])
                                rcph = rcpb_p.tile([64, 256], f32, tag="rcph",
                                                   bufs=2, name="rcph")
                                nc.vector.reciprocal_approx_fast(
                                    out=rcph[0:1, :], in_=den[:])
                                nc.gpsimd.partition_broadcast(rcph[:],
                                                              rcph[0:1, :])
                                nc.vector.tensor_tensor(
                                    attT[64 * e:64 * e + 64, sl],
                                    aps_[e][0:64, sl], rcph[:], op=ALU.mult)
                        continue_tail = True
                    else:
                        continue_tail = False
                    for e, h in enumerate(heads):
                        if continue_tail:
                            break
                        den = rcpb_p.tile([1, 512], f32, tag="den", bufs=2,
                                          name="den")
                        # Scalar is exp-bound in the last phase; DVE has slack
                        # there (no rope work)
                        if lc == 3 or den_v:
                            nc.vector.tensor_copy(den[:], aps_[e][64:65, :])
                        else:
                            nc.scalar.copy(den[:], aps_[e][64:65, :])
                        rcpb = rcpb_p.tile([64, 512], f32, name="rcpb")
                        nc.vector.reciprocal_approx_fast(out=rcpb[0:1, :],
                                                         in_=den[:])
                        nc.gpsimd.partition_broadcast(rcpb[:], rcpb[0:1, :])
                        if debug and lc == 0 and t == 0 and e == 0:
                            nc.sync.dma_start(dbg["rcpb0"][:], rcpb[:])
                        nc.vector.tensor_tensor(
                            attT[64 * e:64 * e + 64, :], aps_[e][0:64, :],
                            rcpb[:], op=ALU.mult)
                while fi < len(fill):
                    fill[fi]()
                    fi += 1

            def outproj_unit(lc, lb, mc, cp="s", pool_at=False):
                """one [128 i, 512 m] output chain: 4 accum mms + drain + DMA"""
                def run():
                    if pool_at:
                        ps = at_ps.tile([128, 512], f32, tag="at", name="ps_o")
                    else:
                        ps = pj_ps.tile([128, 512], f32, tag="pj", name="ps_o")
                    for cb in range(4):
                        nc.tensor.matmul(
                            ps[:],
                            q_tiles[("attT", lc, cb)][:, lb * 128:(lb + 1) * 128],
                            wo_sb[:, cb * 2048 + mc * 512:cb * 2048 + (mc + 1) * 512],
                            start=(cb == 0), stop=(cb == 3))
                    ysb = ysb_p.tile([128, 512], f16 if tweak6 else f32,
                                     name="ysb")
                    if cp == "s":
                        nc.scalar.copy(ysb[:], ps[:])
                    elif cp == "g":
                        nc.gpsimd.tensor_copy(ysb[:], ps[:])
                    else:
                        nc.vector.tensor_copy(ysb[:], ps[:])
                    nc.sync.dma_start(
                        y_d[lc * 512 + lb * 128:lc * 512 + (lb + 1) * 128,
                            mc * 512:(mc + 1) * 512], ysb[:])
                return run

            def outproj_units(lc, cp="s"):
                return [outproj_unit(lc, lb, mc, cp)
                        for lb in range(4) for mc in range(4)]

            # chunk-0 loads with wq-cb1 interleaved so proj cb1 never stalls
            xt0 = []
            if tweak4:
                # big weight transfers first: they claim distinct DMA queues
                # and run in parallel with the small xt tiles queued behind
                nc.sync.dma_start(wq_sb[:, 2048:4096], wq_sb_d[:, 2048:4096])
                nc.sync.dma_start(wq_sb[:, 4096:6144], wq_sb_d[:, 4096:6144])
                nc.sync.dma_start(wq_sb[:, 6144:8192], wq_sb_d[:, 6144:8192])
                nc.sync.dma_start(wk_sb[:], wk_sb_d[:])
                nc.sync.dma_start(wv_sb[:], wv_sb_d[:])
                for db in range(16):
                    t = xt_p.tile([128, 512], f16, tag="xt", name=f"xt{db}")
                    nc.sync.dma_start(t[:], xT[db * 128:(db + 1) * 128, 0:512])
                    xt0.append(t)
                cos_t0 = maps_p.tile([128, 512], mapdt, tag="cos", name="cos_t")
                nc.sync.dma_start(cos_t0[:], cosm_d[:, 0:512])
                sin_t0 = maps_p.tile([128, 512], mapdt, tag="sin", name="sin_t")
                nc.sync.dma_start(sin_t0[:], sinm2_d[:, 0:512])
                nc.sync.dma_start(mask_sb[:], maskT_d[:])
                nc.sync.dma_start(ident_sb[:], ident_d[:])
            elif tweak2:
                for db in range(6):
                    t = xt_p.tile([128, 512], f16, tag="xt", name=f"xt{db}")
                    nc.sync.dma_start(t[:], xT[db * 128:(db + 1) * 128, 0:512])
                    xt0.append(t)
                cos_t0 = maps_p.tile([128, 512], mapdt, tag="cos", name="cos_t")
                nc.sync.dma_start(cos_t0[:], cosm_d[:, 0:512])
                sin_t0 = maps_p.tile([128, 512], mapdt, tag="sin", name="sin_t")
                nc.sync.dma_start(sin_t0[:], sinm2_d[:, 0:512])
                nc.sync.dma_start(wq_sb[:, 2048:4096], wq_sb_d[:, 2048:4096])
                for db in range(6, 12):
                    t = xt_p.tile([128, 512], f16, tag="xt", name=f"xt{db}")
                    nc.sync.dma_start(t[:], xT[db * 128:(db + 1) * 128, 0:512])
                    xt0.append(t)
                nc.sync.dma_start(wk_sb[:], wk_sb_d[:])
                nc.sync.dma_start(wv_sb[:], wv_sb_d[:])
                for db in range(12, 16):
                    t = xt_p.tile([128, 512], f16, tag="xt", name=f"xt{db}")
                    nc.sync.dma_start(t[:], xT[db * 128:(db + 1) * 128, 0:512])
                    xt0.append(t)
            else:
                for db in range(12):
                    t = xt_p.tile([128, 512], f16, tag="xt", name=f"xt{db}")
                    nc.sync.dma_start(t[:], xT[db * 128:(db + 1) * 128, 0:512])
                    xt0.append(t)
                nc.sync.dma_start(wq_sb[:, 2048:4096], wq_sb_d[:, 2048:4096])
                for db in range(12, 16):
                    t = xt_p.tile([128, 512], f16, tag="xt", name=f"xt{db}")
                    nc.sync.dma_start(t[:], xT[db * 128:(db + 1) * 128, 0:512])
                    xt0.append(t)
                cos_t0 = maps_p.tile([128, 512], mapdt, tag="cos", name="cos_t")
                nc.sync.dma_start(cos_t0[:], cosm_d[:, 0:512])
                sin_t0 = maps_p.tile([128, 512], mapdt, tag="sin", name="sin_t")
                nc.sync.dma_start(sin_t0[:], sinm2_d[:, 0:512])
                nc.sync.dma_start(wk_sb[:], wk_sb_d[:])
                nc.sync.dma_start(wv_sb[:], wv_sb_d[:])
            if not tweak4:
                for cb in range(2, 4):
                    nc.sync.dma_start(wq_sb[:, cb * 2048:(cb + 1) * 2048],
                                      wq_sb_d[:, cb * 2048:(cb + 1) * 2048])
                nc.sync.dma_start(mask_sb[:], maskT_d[:])
                nc.sync.dma_start(ident_sb[:], ident_d[:])
            for u in do_proj(0, (xt0, cos_t0, sin_t0)):
                u()
            ld = load_xt(1)
            if not tweak:
                nc.sync.dma_start(wo_sb[:], wo_sb_d[:])
            if weave:
                # fillers woven into the att jb loops
                do_att(0, fill=do_proj(1, ld))
                if tweak:
                    # wo is first needed by op(0) in phase 1; loading it
                    # after phase 0 keeps startup DMA queues for xt/kz
                    for c in range(4):
                        nc.sync.dma_start(wo_sb[:, c * 2048:(c + 1) * 2048],
                                          wo_sb_d[:, c * 2048:(c + 1) * 2048])
                ld = load_xt(2)
                p2 = do_proj(2, ld)
                op0 = outproj_units(0, cp="g" if ysbg else "s")
                do_att(1, fill=p2[:4] + op0[:4] + p2[4:5] + op0[4:8]
                              + p2[5:6] + op0[8:16])
                ld = load_xt(3)
                p3 = do_proj(3, ld)
                op1a = [outproj_unit(1, lb, mc, "g" if ysbg else "s")
                        for lb in range(3) for mc in range(4)]
                op1b = [outproj_unit(1, 3, mc, "v") for mc in range(4)]
                do_att(2, fill=p3[:4] + op1a[:4] + p3[4:5] + op1a[4:8]
                              + p3[5:6] + op1a[8:12])
                op2 = outproj_units(2, cp="v")
                do_att(3, fill=op1b + op2)
            else:
                # block emission: att first (priority), filler chains after;
                # the scheduler pulls whole chains in when att stalls
                do_att(0)
                for u in do_proj(1, ld):
                    u()
                ld = load_xt(2)
                for u in outproj_units(0, cp="s"):
                    u()
                do_att(1)
                for u in do_proj(2, ld):
                    u()
                ld = load_xt(3)
                do_att(2, pairs=(0, 1))
                for u in outproj_units(1, cp="s"):
                    u()
                do_att(2, pairs=(2, 3))
                for u in do_proj(3, ld):
                    u()
                do_att(3, pairs=(0, 1))
                for u in outproj_units(2, cp="v"):
                    u()
                do_att(3, pairs=(2, 3))
            # tail: att psum bufs are free, run 4 chains in flight
            for i in range(16):
                outproj_unit(3, i // 4, i % 4, "s" if i % 2 == 0 else "v",
                             pool_at=(i % 2 == 1))()

            if debug:
                nc.sync.dma_start(dbg["q00"][:], q_tiles[(0, 0)][:])
                nc.sync.dma_start(dbg["kz00"][:], kz[0][0][:])
                nc.sync.dma_start(dbg["vext0"][:], vext[0][:])
                nc.sync.dma_start(dbg["attT00"][:], q_tiles[("attT", 0, 0)][:])

    nc.compile()
    return nc


def _perm64(w):
    # per 64-row head block: [te 0:16 | to 0:16 | te 16:32 | to 16:32]
    # so the RoPE partner is a 16-row swap inside each 32-partition quadrant
    e, o = w[0::2], w[1::2]
    return np.concatenate([e[0:16], o[0:16], e[16:32], o[16:32]], axis=0)


# row r of a 64-row head block holds RoPE pair _PAIR_IDX64[r]
_PAIR_IDX64 = np.concatenate(
    [np.r_[0:16], np.r_[0:16], np.r_[16:32], np.r_[16:32]])
_PAIR_IDX128 = np.concatenate([_PAIR_IDX64, _PAIR_IDX64])


def _prep_core_inputs(core, x, wq, wk, wv, wo, fc, fs, mask, xT_cache,
                      map_dt=np.float32, ident_dt=np.float16):
    b, g = divmod(core, 4)
    hq0 = 8 * g
    if b not in xT_cache:
        xT_cache[b] = np.ascontiguousarray(x[b].T, dtype=np.float16)
    xT = xT_cache[b]

    wq_s = (wq[hq0 * 64:(hq0 + 8) * 64] * SCALE).astype(F32)
    wq_p = np.concatenate([_perm64(wq_s[h * 64:(h + 1) * 64]) for h in range(8)], 0)
    wqT = wq_p.T  # [D, 512]
    # wq_sb[p, (cb*16+db)*128 + c] = wqT[db*128+p, cb*128+c]
    wq_sb = np.ascontiguousarray(
        wqT.reshape(16, 128, 4, 128).transpose(1, 2, 0, 3).reshape(128, 8192),
        dtype=np.float16)

    wk_s = wk[2 * g * 64:(2 * g + 2) * 64]
    wk_p = np.concatenate([_perm64(wk_s[h * 64:(h + 1) * 64]) for h in range(2)], 0)
    wkT = wk_p.T  # [D, 128]
    wk_sb = np.ascontiguousarray(
        wkT.reshape(16, 128, 128).transpose(1, 0, 2).reshape(128, 2048),
        dtype=np.float16)

    wvT = wv[2 * g * 64:(2 * g + 2) * 64].T  # [D, 128]
    wv_sb = np.ascontiguousarray(
        wvT.reshape(16, 128, 128).transpose(1, 0, 2).reshape(128, 2048),
        dtype=np.float16)

    woT = wo[:, hq0 * 64:(hq0 + 8) * 64].T  # [512, D]
    wo_sb = np.ascontiguousarray(
        woT.reshape(4, 128, 4, 512).transpose(1, 0, 2, 3).reshape(128, 8192),
        dtype=np.float16)

    cosT = fc.T[_PAIR_IDX128].astype(F32)  # [128, L]
    sinT = fs.T[_PAIR_IDX128].astype(F32)
    sgn = np.ones((128, 1), F32)
    for q in range(4):
        sgn[q * 32 + 16:q * 32 + 32] = -1  # to-rows carry -sin
    sinm2 = np.ascontiguousarray(sinT * sgn)

    maskT = np.ascontiguousarray(mask[0, 0, :128, :128].T, dtype=F32)

    return {"xT": xT, "wq_sb": wq_sb, "wk_sb": wk_sb, "wv_sb": wv_sb,
            "wo_sb": wo_sb,
            "cosm": np.ascontiguousarray(cosT.astype(map_dt)),
            "sinm2": np.ascontiguousarray(sinm2.astype(map_dt)),
            "maskT": maskT, "ident": np.eye(128, dtype=ident_dt)}


def kernel(x, wq, wk, wv, wo, freqs_cos, freqs_sin, mask):
    from concourse import bass_utils

    if "nc" not in _CACHE:
        _CACHE["nc"] = _build_nc(weave=True, tweak=True, den_v=True,
                                 tweak2=True, tweak6=True,
                                 warm2=18, trf16=True, dennorm=False)
    nc = _CACHE["nc"]

    x = np.asarray(x, F32)
    xT_cache = {}
    in_maps = [
        _prep_core_inputs(c, x, np.asarray(wq, F32), np.asarray(wk, F32),
                          np.asarray(wv, F32), np.asarray(wo, F32),
                          np.asarray(freqs_cos, F32), np.asarray(freqs_sin, F32),
                          np.asarray(mask, F32), xT_cache)
        for c in range(NCORES)
    ]
    res = bass_utils.run_bass_kernel_spmd(nc, in_maps, core_ids=list(range(NCORES)))
    out = np.zeros((B, L, D), F32)
    for c in range(NCORES):
        out[c // 4] += res.results[c]["y"]
    return out



# revision 17
# speedup vs baseline: 1.0019x; 1.0019x over previous
"""Trainium2 Bass kernel for nn_Attention_4449586119407.

GQA attention layer (B=2, L=2048, D=2048, 32 Q heads / 8 KV heads, RoPE,
causal) sharded over 8 NeuronCores: data-parallel over batch (2) x
tensor-parallel over heads (4 groups of 8 Q heads / 2 KV heads).
wq/wk/wv column-sharded, wo row-sharded; the wo partial sums are reduced
on the host.

Device-side layout (per core):
  - All matmuls contract over the SBUF partition dim.  x is pre-transposed
    on the host (xT [D, L]) so QKV projections produce channel-major
    qT/kT [c, l] directly.
  - RoPE pairs are made partition-contiguous by permuting wq/wk rows on the
    host (per head: even rows then odd rows -> [te(32), to(32)] blocks).
    The rotation partner (partition swap te<->to) is produced with 4 small
    SBUF->SBUF DMAs; the rotation itself is 3 full-width DVE ops using
    host-precomputed cos / signed-sin maps.
  - Scores are computed transposed (S[j, i] = k . q) so the softmaxed tile
    can feed the P@V matmul directly as the stationary operand.  Softmax
    skips the max-subtraction (logits are O(5) here) and gets the
    denominator for free as a 65th "ones" column of V.
  - Causality is structural: only lower-triangle j-blocks are computed and
    the 128x128 diagonal blocks get the (transposed) mask block added in
    PSUM before the exp.
"""

import numpy as np

B, L, D = 2, 2048, 2048
NH, NKV, HD = 32, 8, 64
SCALE = HD ** -0.5
NCORES = 8
F32 = np.float32
SHUF_MASK = [(i + 16) % 32 for i in range(32)]

_CACHE = {}


def _build_nc(debug=False, weave=False, tweak=False, den_v=False,
              tweak2=False, tweak3=False, tweak4=False, tweak5=False,
              tweak6=False, tweak7=False, tweak8=False,
              warm2=0, trf16=False, dennorm=False, vtg=False, ysbg=False):
    from contextlib import ExitStack

    import concourse.tile as tile
    from concourse import bacc, mybir

    f32 = mybir.dt.float32
    f32r = mybir.dt.float32r
    f16 = mybir.dt.float16
    AF = mybir.ActivationFunctionType
    ALU = mybir.AluOpType

    nc = bacc.Bacc("TRN2", target_bir_lowering=False, debug=False,
                   num_devices=NCORES)

    xT = nc.dram_tensor("xT", [D, L], f16, kind="ExternalInput").ap()
    wq_sb_d = nc.dram_tensor("wq_sb", [128, 8192], f16, kind="ExternalInput").ap()
    wk_sb_d = nc.dram_tensor("wk_sb", [128, 2048], f16, kind="ExternalInput").ap()
    wv_sb_d = nc.dram_tensor("wv_sb", [128, 2048], f16, kind="ExternalInput").ap()
    wo_sb_d = nc.dram_tensor("wo_sb", [128, 8192], f16, kind="ExternalInput").ap()
    mapdt = f16 if tweak7 else f32
    cosm_d = nc.dram_tensor("cosm", [128, 2048], mapdt, kind="ExternalInput").ap()
    sinm2_d = nc.dram_tensor("sinm2", [128, 2048], mapdt, kind="ExternalInput").ap()
    maskT_d = nc.dram_tensor("maskT", [128, 128], f32, kind="ExternalInput").ap()
    identdt = f16 if trf16 else f32
    ident_d = nc.dram_tensor("ident", [128, 128], identdt, kind="ExternalInput").ap()
    y_d = nc.dram_tensor("y", [L, D], f16 if tweak6 else f32,
                         kind="ExternalOutput").ap()
    if debug:
        dbg = {nm: nc.dram_tensor(f"dbg_{nm}", sh, dt, kind="ExternalOutput").ap()
               for nm, sh, dt in [
                   ("q00", [128, 512], f16), ("kz00", [128, 2048], f16),
                   ("vext0", [128, 2048], f16), ("p000", [128, 1024], f16),
                   ("attT00", [128, 512], f16), ("att0", [128, 512], f32), ("rcpb0", [64, 512], f32)]}

    with tile.TileContext(nc) as tc:
        with ExitStack() as ctx:
            singles = ctx.enter_context(tc.tile_pool(name="singles", bufs=1))
            xt_p = ctx.enter_context(tc.tile_pool(name="xt", bufs=24))
            maps_p = ctx.enter_context(tc.tile_pool(name="maps", bufs=4))
            qrot_p = ctx.enter_context(tc.tile_pool(name="qrot", bufs=8))
            ptile_p = ctx.enter_context(tc.tile_pool(name="pt", bufs=4))
            tmp_p = ctx.enter_context(tc.tile_pool(name="tmp", bufs=3))
            attT_p = ctx.enter_context(tc.tile_pool(name="attT", bufs=13))
            rcpb_p = ctx.enter_context(tc.tile_pool(name="rcpb", bufs=2))
            ysb_p = ctx.enter_context(tc.tile_pool(name="ysb",
                                                   bufs=8 if tweak8 else 3))
            pj_ps = ctx.enter_context(tc.tile_pool(name="pj", bufs=2, space="PSUM"))
            sc_ps = ctx.enter_context(tc.tile_pool(name="sc", bufs=2, space="PSUM"))
            at_ps = ctx.enter_context(tc.tile_pool(name="at", bufs=2, space="PSUM"))

            # ---- HAM warmup: keep PE busy while the first DMAs land ----
            if warm2:
                # 512-row warm matmuls sized to end as xt chunk 0 lands, with
                # the PE clock fully ramped (LOW 100ns -> MID 3us -> FULL)
                warm_sb = singles.tile([128, 512], f16)
                nc.vector.memset(warm_sb[:], 0.0)
                warm_ps = pj_ps.tile([64, 512], f32, tag="pj", name="warm_ps")
                for _ in range(warm2):
                    nc.tensor.matmul(warm_ps[:], warm_sb[:, 0:64], warm_sb[:],
                                     start=True, stop=True)
            else:
                warm_sb = singles.tile([128, 128], f16)
                nc.vector.memset(warm_sb[:], 0.0)
                warm_ps = pj_ps.tile([64, 64], f32, tag="pj", name="warm_ps")
                for _ in range(30 if tweak2 else 90):
                    nc.tensor.matmul(warm_ps[:], warm_sb[:, 0:64],
                                     warm_sb[:, 0:64], start=True, stop=True)

            # ---- resident constants (wq cb0 first; bulk after xt chunk 0) ----
            # cb0 split into 512-col chunks so the first q-chain stationaries
            # land sooner
            wq_sb = singles.tile([128, 8192], f16)
            for c in range(4):
                nc.sync.dma_start(wq_sb[:, c * 512:(c + 1) * 512],
                                  wq_sb_d[:, c * 512:(c + 1) * 512])
            wk_sb = singles.tile([128, 2048], f16)
            wv_sb = singles.tile([128, 2048], f16)
            mask_sb = singles.tile([128, 128], f32)
            ident_sb = singles.tile([128, 128], identdt)
            wo_sb = singles.tile([128, 8192], f16)

            def load_weights_bulk():
                for cb in range(1, 4):
                    nc.sync.dma_start(wq_sb[:, cb * 2048:(cb + 1) * 2048],
                                      wq_sb_d[:, cb * 2048:(cb + 1) * 2048])
                nc.sync.dma_start(wk_sb[:], wk_sb_d[:])
                nc.sync.dma_start(wv_sb[:], wv_sb_d[:])
                nc.sync.dma_start(mask_sb[:], maskT_d[:])
                nc.sync.dma_start(ident_sb[:], ident_d[:])
            # kz[kv][half]: rotated k for kv head, in partition half `half`,
            # other half zero -> K=128 scores matmuls with full partitions
            kz = [[singles.tile([128, 2048], f16, name=f"kz{kv}{hf}")
                   for hf in range(2)] for kv in range(2)]
            for kv in range(2):
                # only the pad half must be zero; the data half is written
                nc.vector.memset(kz[kv][0][64:128, :], 0.0)
                nc.vector.memset(kz[kv][1][0:64, :], 0.0)
            # per jb: [v(64) | ones(1) | pad(63)] -> full 128-col stationary;
            # with dennorm the pad becomes 64 ones-columns so the PV matmul
            # replicates the softmax denominator into psum partitions 64-127
            # (normalize then needs no partition broadcast)
            vext = [singles.tile([128, 2048], f16, name=f"vext{kv}")
                    for kv in range(2)]
            for kv in range(2):
                if dennorm:
                    v3d = vext[kv].rearrange("p (j c) -> p j c", c=128)
                    nc.vector.memset(v3d[:, :, 64:128], 1.0)
                else:
                    nc.vector.memset(vext[kv][:, 64::128], 1.0)

            def diag_off(jb, lc):
                """col offset of the diagonal 128-block inside chunk lc, or None"""
                od = 128 * jb - 512 * lc
                return od if 0 <= od < 512 else None

            q_tiles = {}

            def load_xt(lc):
                lsl = slice(lc * 512, (lc + 1) * 512)
                xt = []
                for db in range(16):
                    t = xt_p.tile([128, 512], f16, tag="xt", name=f"xt{db}")
                    nc.sync.dma_start(t[:], xT[db * 128:(db + 1) * 128, lsl])
                    xt.append(t)
                cos_t = maps_p.tile([128, 512], mapdt, tag="cos", name="cos_t")
                nc.sync.dma_start(cos_t[:], cosm_d[:, lsl])
                sin_t = maps_p.tile([128, 512], mapdt, tag="sin", name="sin_t")
                nc.sync.dma_start(sin_t[:], sinm2_d[:, lsl])
                return xt, cos_t, sin_t

            def do_proj(lc, loaded):
                lsl = slice(lc * 512, (lc + 1) * 512)
                xt, cos_t, sin_t = loaded

                def rope(ps, dest):
                    """rotate [128,512] psum block into dest (SBUF).

                    RoPE pairs are laid out [te(16) | to(16)] per 32-partition
                    quadrant (host-side row perm), so the rotation partner is
                    an intra-quadrant 16-block swap -> one DVE stream_shuffle.
                    """
                    t2 = tmp_p.tile([128, 512], f32, tag="t2", bufs=2, name="t2")
                    nc.vector.tensor_tensor(t2[:], ps[:], sin_t[:], op=ALU.mult)
                    qsw = tmp_p.tile([128, 512], f32, tag="qsw", name="qsw")
                    nc.vector.stream_shuffle(qsw[:], t2[:], mask=SHUF_MASK)
                    t3 = tmp_p.tile([128, 512], f32, tag="t3", bufs=2, name="t3")
                    nc.vector.tensor_tensor(t3[:], ps[:], cos_t[:], op=ALU.mult)
                    nc.vector.tensor_tensor(dest[:], t3[:], qsw[:], op=ALU.add)

                def q_unit(cb):
                    ps = pj_ps.tile([128, 512], f32, tag="pj", name="ps_q")
                    for db in range(16):
                        nc.tensor.matmul(
                            ps[:], wq_sb[:, (cb * 16 + db) * 128:(cb * 16 + db + 1) * 128],
                            xt[db][:], start=(db == 0), stop=(db == 15))
                    qt = qrot_p.tile([128, 512], f16, name="qt")
                    rope(ps, qt)
                    q_tiles[(cb, lc)] = qt

                def k_unit():
                    # k projection + RoPE + zero-padded scatter
                    ps = pj_ps.tile([128, 512], f32, tag="pj", name="ps_k")
                    for db in range(16):
                        nc.tensor.matmul(
                            ps[:], wk_sb[:, db * 128:(db + 1) * 128],
                            xt[db][:], start=(db == 0), stop=(db == 15))
                    kraw = tmp_p.tile([128, 512], f16, tag="kraw", bufs=2,
                                      name="kraw")
                    rope(ps, kraw)
                    for kv in range(2):
                        nc.sync.dma_start(kz[kv][0][0:64, lsl],
                                          kraw[kv * 64:kv * 64 + 64, :])
                        nc.sync.dma_start(kz[kv][1][64:128, lsl],
                                          kraw[kv * 64:kv * 64 + 64, :])

                def v_unit():
                    # v projection (transposed) then PE-transpose per block
                    vtdt = f16 if trf16 else f32
                    vt = tmp_p.tile([128, 512], vtdt, tag="vt", bufs=2,
                                    name="vt")
                    ps = pj_ps.tile([128, 512], f32, tag="pj", name="ps_v")
                    for db in range(16):
                        nc.tensor.matmul(
                            ps[:], wv_sb[:, db * 128:(db + 1) * 128],
                            xt[db][:], start=(db == 0), stop=(db == 15))
                    if vtg:
                        # GpSimd is idle and vext isn't needed until the next
                        # att phase; keep Scalar free for exp, DVE for rope
                        nc.gpsimd.tensor_copy(vt[:], ps[:])
                    else:
                        nc.scalar.copy(vt[:], ps[:])
                    for j in range(4):
                        jb = 4 * lc + j
                        ps = pj_ps.tile([128, 128], vtdt if trf16 else f32,
                                        tag="pj", name="ps_t")
                        nc.tensor.transpose(ps[:], vt[:, j * 128:(j + 1) * 128],
                                            ident_sb[:])
                        for kv in range(2):
                            nc.vector.tensor_copy(
                                vext[kv][:, jb * 128:jb * 128 + 64],
                                ps[:, kv * 64:kv * 64 + 64])

                # first att pair needs q-cb0 + kz + vext: emit those first
                return [lambda: q_unit(0), k_unit, v_unit,
                        lambda: q_unit(1), lambda: q_unit(2),
                        lambda: q_unit(3)]

            def do_att(lc, pairs=(0, 1, 2, 3), fill=()):
                # fill: closures (proj/outproj chains) woven between jb
                # iterations so the PE always has ready work while exp runs
                fill = list(fill)
                fi = 0
                acc = 0.0
                njb = 4 * lc + 4
                per = len(fill) / (len(pairs) * njb) if fill else 0.0
                for t in pairs:
                    heads = (2 * t, 2 * t + 1)
                    aps_ = [at_ps.tile([128, 512], f32, tag="at", name=f"at{e}")
                            for e in range(2)]
                    P = None
                    for jb in range(njb):
                        o = max(0, 128 * jb - 512 * lc)
                        S = sc_ps.tile([128, 1024], f32, tag="sc", name="S")
                        for e, h in enumerate(heads):
                            kt = kz[h // 4][h % 2]
                            nc.tensor.matmul(
                                S[:, e * 512 + o:(e + 1) * 512],
                                kt[:, jb * 128:(jb + 1) * 128],
                                q_tiles[(h // 2, lc)][:, o:512],
                                start=True, stop=True)
                        od = diag_off(jb, lc)
                        if od is not None and tweak3:
                            for e in range(2):
                                sl = slice(e * 512 + od, e * 512 + od + 128)
                                nc.vector.tensor_tensor(S[:, sl], S[:, sl],
                                                        mask_sb[:], op=ALU.add)
                        elif od is not None:
                            s3 = S.rearrange("p (e c) -> p e c", e=2)[:, :, od:od + 128]
                            m3 = mask_sb[:].unsqueeze(1).broadcast_to([128, 2, 128])
                            nc.vector.tensor_tensor(s3, s3, m3, op=ALU.add)
                        P = ptile_p.tile([128, 1024], f16, name="P")
                        if o == 0:
                            nc.scalar.activation(P[:], S[:], AF.Exp)
                        elif tweak3:
                            for e in range(2):
                                sl = slice(e * 512 + o, (e + 1) * 512)
                                nc.scalar.activation(P[:, sl], S[:, sl], AF.Exp)
                        else:
                            s3 = S.rearrange("p (e c) -> p e c", e=2)[:, :, o:512]
                            p3 = P.rearrange("p (e c) -> p e c", e=2)[:, :, o:512]
                            nc.scalar.activation(p3, s3, AF.Exp)
                        for e, h in enumerate(heads):
                            kv = h // 4
                            nc.tensor.matmul(
                                aps_[e][:, o:512],
                                vext[kv][:, jb * 128:jb * 128 + 128],
                                P[:, e * 512 + o:(e + 1) * 512],
                                start=(jb == 0), stop=(jb == njb - 1),
                                skip_group_check=True)
                        acc += per
                        while fi < len(fill) and fi < int(acc + 1e-9):
                            fill[fi]()
                            fi += 1
                    if debug and lc == 0 and t == 0:
                        nc.sync.dma_start(dbg["p000"][:], P[:])
                    attT = attT_p.tile([128, 512], f16, name="attT")
                    q_tiles[("attT", lc, t)] = attT
                    if debug and lc == 0 and t == 0:
                        dbga = tmp_p.tile([128, 512], f32, tag="dbga", bufs=1,
                                          name="dbga")
                        nc.vector.tensor_copy(dbga[:], aps_[0][:])
                        nc.sync.dma_start(dbg["att0"][:], dbga[:])
                    if dennorm:
                        # psum rows 64-127 hold the denominator replicated by
                        # the ones-columns of vext: copy-align + reciprocal +
                        # multiply, all DVE, no partition broadcast.
                        # (reciprocal_approx_fast ignores input partition
                        # offsets, so the den block is first copied to a
                        # base-0 window; tensor_copy does honor the shift.)
                        for e in range(2):
                            den64 = rcpb_p.tile([64, 512], f32, tag="den64",
                                                name="den64")
                            nc.vector.tensor_copy(den64[:],
                                                  aps_[e][64:128, :])
                            rcp = rcpb_p.tile([64, 512], f32, tag="rcp",
                                              name="rcp")
                            nc.vector.reciprocal_approx_fast(
                                out=rcp[:], in_=den64[:])
                            nc.vector.tensor_tensor(
                                attT[64 * e:64 * e + 64, :], aps_[e][0:64, :],
                                rcp[:], op=ALU.mult)
                        continue
                    if tweak5 and lc == 3 and t == 3:
                        # last pair gates the outproj(3) tail: normalize in
                        # i-halves, both heads' first half first, so lb0/1
                        # output chains start while the second half drains
                        for hh in range(2):
                            sl = slice(hh * 256, (hh + 1) * 256)
                            for e in range(2):
                                den = rcpb_p.tile([1, 256], f32, tag="denh",
                                                  bufs=2, name="denh")
                                nc.vector.tensor_copy(den[:],
                                                      aps_[e][64:65, sl])
                                rcph = rcpb_p.tile([64, 256], f32, tag="rcph",
                                                   bufs=2, name="rcph")
                                nc.vector.reciprocal_approx_fast(
                                    out=rcph[0:1, :], in_=den[:])
                                nc.gpsimd.partition_broadcast(rcph[:],
                                                              rcph[0:1, :])
                                nc.vector.tensor_tensor(
                                    attT[64 * e:64 * e + 64, sl],
                                    aps_[e][0:64, sl], rcph[:], op=ALU.mult)
                        continue_tail = True
                    else:
                        continue_tail = False
                    for e, h in enumerate(heads):
                        if continue_tail:
                            break
                        den = rcpb_p.tile([1, 512], f32, tag="den", bufs=2,
                                          name="den")
                        # Scalar is exp-bound in the last phase; DVE has slack
                        # there (no rope work)
                        if lc == 3 or den_v:
                            nc.vector.tensor_copy(den[:], aps_[e][64:65, :])
                        else:
                            nc.scalar.copy(den[:], aps_[e][64:65, :])
                        rcpb = rcpb_p.tile([64, 512], f32, name="rcpb")
                        nc.vector.reciprocal_approx_fast(out=rcpb[0:1, :],
                                                         in_=den[:])
                        nc.gpsimd.partition_broadcast(rcpb[:], rcpb[0:1, :])
                        if debug and lc == 0 and t == 0 and e == 0:
                            nc.sync.dma_start(dbg["rcpb0"][:], rcpb[:])
                        nc.vector.tensor_tensor(
                            attT[64 * e:64 * e + 64, :], aps_[e][0:64, :],
                            rcpb[:], op=ALU.mult)
                while fi < len(fill):
                    fill[fi]()
                    fi += 1

            def outproj_unit(lc, lb, mc, cp="s", pool_at=False):
                """one [128 i, 512 m] output chain: 4 accum mms + drain + DMA"""
                def run():
                    if pool_at:
                        ps = at_ps.tile([128, 512], f32, tag="at", name="ps_o")
                    else:
                        ps = pj_ps.tile([128, 512], f32, tag="pj", name="ps_o")
                    for cb in range(4):
                        nc.tensor.matmul(
                            ps[:],
                            q_tiles[("attT", lc, cb)][:, lb * 128:(lb + 1) * 128],
                            wo_sb[:, cb * 2048 + mc * 512:cb * 2048 + (mc + 1) * 512],
                            start=(cb == 0), stop=(cb == 3))
                    ysb = ysb_p.tile([128, 512], f16 if tweak6 else f32,
                                     name="ysb")
                    if cp == "s":
                        nc.scalar.copy(ysb[:], ps[:])
                    elif cp == "g":
                        nc.gpsimd.tensor_copy(ysb[:], ps[:])
                    else:
                        nc.vector.tensor_copy(ysb[:], ps[:])
                    nc.sync.dma_start(
                        y_d[lc * 512 + lb * 128:lc * 512 + (lb + 1) * 128,
                            mc * 512:(mc + 1) * 512], ysb[:])
                return run

            def outproj_units(lc, cp="s"):
                return [outproj_unit(lc, lb, mc, cp)
                        for lb in range(4) for mc in range(4)]

            # chunk-0 loads with wq-cb1 interleaved so proj cb1 never stalls
            xt0 = []
            if tweak4:
                # big weight transfers first: they claim distinct DMA queues
                # and run in parallel with the small xt tiles queued behind
                nc.sync.dma_start(wq_sb[:, 2048:4096], wq_sb_d[:, 2048:4096])
                nc.sync.dma_start(wq_sb[:, 4096:6144], wq_sb_d[:, 4096:6144])
                nc.sync.dma_start(wq_sb[:, 6144:8192], wq_sb_d[:, 6144:8192])
                nc.sync.dma_start(wk_sb[:], wk_sb_d[:])
                nc.sync.dma_start(wv_sb[:], wv_sb_d[:])
                for db in range(16):
                    t = xt_p.tile([128, 512], f16, tag="xt", name=f"xt{db}")
                    nc.sync.dma_start(t[:], xT[db * 128:(db + 1) * 128, 0:512])
                    xt0.append(t)
                cos_t0 = maps_p.tile([128, 512], mapdt, tag="cos", name="cos_t")
                nc.sync.dma_start(cos_t0[:], cosm_d[:, 0:512])
                sin_t0 = maps_p.tile([128, 512], mapdt, tag="sin", name="sin_t")
                nc.sync.dma_start(sin_t0[:], sinm2_d[:, 0:512])
                nc.sync.dma_start(mask_sb[:], maskT_d[:])
                nc.sync.dma_start(ident_sb[:], ident_d[:])
            elif tweak2:
                for db in range(6):
                    t = xt_p.tile([128, 512], f16, tag="xt", name=f"xt{db}")
                    nc.sync.dma_start(t[:], xT[db * 128:(db + 1) * 128, 0:512])
                    xt0.append(t)
                cos_t0 = maps_p.tile([128, 512], mapdt, tag="cos", name="cos_t")
                nc.sync.dma_start(cos_t0[:], cosm_d[:, 0:512])
                sin_t0 = maps_p.tile([128, 512], mapdt, tag="sin", name="sin_t")
                nc.sync.dma_start(sin_t0[:], sinm2_d[:, 0:512])
                nc.sync.dma_start(wq_sb[:, 2048:4096], wq_sb_d[:, 2048:4096])
                for db in range(6, 12):
                    t = xt_p.tile([128, 512], f16, tag="xt", name=f"xt{db}")
                    nc.sync.dma_start(t[:], xT[db * 128:(db + 1) * 128, 0:512])
                    xt0.append(t)
                nc.sync.dma_start(wk_sb[:], wk_sb_d[:])
                nc.sync.dma_start(wv_sb[:], wv_sb_d[:])
                for db in range(12, 16):
                    t = xt_p.tile([128, 512], f16, tag="xt", name=f"xt{db}")
                    nc.sync.dma_start(t[:], xT[db * 128:(db + 1) * 128, 0:512])
                    xt0.append(t)
            else:
                for db in range(12):
                    t = xt_p.tile([128, 512], f16, tag="xt", name=f"xt{db}")
                    nc.sync.dma_start(t[:], xT[db * 128:(db + 1) * 128, 0:512])
                    xt0.append(t)
                nc.sync.dma_start(wq_sb[:, 2048:4096], wq_sb_d[:, 2048:4096])
                for db in range(12, 16):
                    t = xt_p.tile([128, 512], f16, tag="xt", name=f"xt{db}")
                    nc.sync.dma_start(t[:], xT[db * 128:(db + 1) * 128, 0:512])
                    xt0.append(t)
                cos_t0 = maps_p.tile([128, 512], mapdt, tag="cos", name="cos_t")
                nc.sync.dma_start(cos_t0[:], cosm_d[:, 0:512])
                sin_t0 = maps_p.tile([128, 512], mapdt, tag="sin", name="sin_t")
                nc.sync.dma_start(sin_t0[:], sinm2_d[:, 0:512])
                nc.sync.dma_start(wk_sb[:], wk_sb_d[:])
                nc.sync.dma_start(wv_sb[:], wv_sb_d[:])
            if not tweak4:
                for cb in range(2, 4):
                    nc.sync.dma_start(wq_sb[:, cb * 2048:(cb + 1) * 2048],
                                      wq_sb_d[:, cb * 2048:(cb + 1) * 2048])
                nc.sync.dma_start(mask_sb[:], maskT_d[:])
                nc.sync.dma_start(ident_sb[:], ident_d[:])
            for u in do_proj(0, (xt0, cos_t0, sin_t0)):
                u()
            ld = load_xt(1)
            if not tweak:
                nc.sync.dma_start(wo_sb[:], wo_sb_d[:])
            if weave:
                # fillers woven into the att jb loops
                do_att(0, fill=do_proj(1, ld))
                if tweak:
                    # wo is first needed by op(0) in phase 1; loading it
                    # after phase 0 keeps startup DMA queues for xt/kz
                    for c in range(4):
                        nc.sync.dma_start(wo_sb[:, c * 2048:(c + 1) * 2048],
                                          wo_sb_d[:, c * 2048:(c + 1) * 2048])
                ld = load_xt(2)
                p2 = do_proj(2, ld)
                op0 = outproj_units(0, cp="g" if ysbg else "s")
                do_att(1, fill=p2[:4] + op0[:4] + p2[4:5] + op0[4:8]
                              + p2[5:6] + op0[8:16])
                ld = load_xt(3)
                p3 = do_proj(3, ld)
                op1a = [outproj_unit(1, lb, mc, "g" if ysbg else "s")
                        for lb in range(3) for mc in range(4)]
                op1b = [outproj_unit(1, 3, mc, "v") for mc in range(4)]
                do_att(2, fill=p3[:4] + op1a[:4] + p3[4:5] + op1a[4:8]
                              + p3[5:6] + op1a[8:12])
                op2 = outproj_units(2, cp="v")
                do_att(3, fill=op1b + op2)
            else:
                # block emission: att first (priority), filler chains after;
                # the scheduler pulls whole chains in when att stalls
                do_att(0)
                for u in do_proj(1, ld):
                    u()
                ld = load_xt(2)
                for u in outproj_units(0, cp="s"):
                    u()
                do_att(1)
                for u in do_proj(2, ld):
                    u()
                ld = load_xt(3)
                do_att(2, pairs=(0, 1))
                for u in outproj_units(1, cp="s"):
                    u()
                do_att(2, pairs=(2, 3))
                for u in do_proj(3, ld):
                    u()
                do_att(3, pairs=(0, 1))
                for u in outproj_units(2, cp="v"):
                    u()
                do_att(3, pairs=(2, 3))
            # tail: att psum bufs are free, run 4 chains in flight
            for i in range(16):
                outproj_unit(3, i // 4, i % 4, "s" if i % 2 == 0 else "v",
                             pool_at=(i % 2 == 1))()

            if debug:
                nc.sync.dma_start(dbg["q00"][:], q_tiles[(0, 0)][:])
                nc.sync.dma_start(dbg["kz00"][:], kz[0][0][:])
                nc.sync.dma_start(dbg["vext0"][:], vext[0][:])
                nc.sync.dma_start(dbg["attT00"][:], q_tiles[("attT", 0, 0)][:])

    nc.compile()
    return nc


def _perm64(w):
    # per 64-row head block: [te 0:16 | to 0:16 | te 16:32 | to 16:32]
    # so the RoPE partner is a 16-row swap inside each 32-partition quadrant
    e, o = w[0::2], w[1::2]
    return np.concatenate([e[0:16], o[0:16], e[16:32], o[16:32]], axis=0)


# row r of a 64-row head block holds RoPE pair _PAIR_IDX64[r]
_PAIR_IDX64 = np.concatenate(
    [np.r_[0:16], np.r_[0:16], np.r_[16:32], np.r_[16:32]])
_PAIR_IDX128 = np.concatenate([_PAIR_IDX64, _PAIR_IDX64])


def _prep_core_inputs(core, x, wq, wk, wv, wo, fc, fs, mask, xT_cache,
                      map_dt=np.float32, ident_dt=np.float16):
    b, g = divmod(core, 4)
    hq0 = 8 * g
    if b not in xT_cache:
        xT_cache[b] = np.ascontiguousarray(x[b].T, dtype=np.float16)
    xT = xT_cache[b]

    wq_s = (wq[hq0 * 64:(hq0 + 8) * 64] * SCALE).astype(F32)
    wq_p = np.concatenate([_perm64(wq_s[h * 64:(h + 1) * 64]) for h in range(8)], 0)
    wqT = wq_p.T  # [D, 512]
    # wq_sb[p, (cb*16+db)*128 + c] = wqT[db*128+p, cb*128+c]
    wq_sb = np.ascontiguousarray(
        wqT.reshape(16, 128, 4, 128).transpose(1, 2, 0, 3).reshape(128, 8192),
        dtype=np.float16)

    wk_s = wk[2 * g * 64:(2 * g + 2) * 64]
    wk_p = np.concatenate([_perm64(wk_s[h * 64:(h + 1) * 64]) for h in range(2)], 0)
    wkT = wk_p.T  # [D, 128]
    wk_sb = np.ascontiguousarray(
        wkT.reshape(16, 128, 128).transpose(1, 0, 2).reshape(128, 2048),
        dtype=np.float16)

    wvT = wv[2 * g * 64:(2 * g + 2) * 64].T  # [D, 128]
    wv_sb = np.ascontiguousarray(
        wvT.reshape(16, 128, 128).transpose(1, 0, 2).reshape(128, 2048),
        dtype=np.float16)

    woT = wo[:, hq0 * 64:(hq0 + 8) * 64].T  # [512, D]
    wo_sb = np.ascontiguousarray(
        woT.reshape(4, 128, 4, 512).transpose(1, 0, 2, 3).reshape(128, 8192),
        dtype=np.float16)

    cosT = fc.T[_PAIR_IDX128].astype(F32)  # [128, L]
    sinT = fs.T[_PAIR_IDX128].astype(F32)
    sgn = np.ones((128, 1), F32)
    for q in range(4):
        sgn[q * 32 + 16:q * 32 + 32] = -1  # to-rows carry -sin
    sinm2 = np.ascontiguousarray(sinT * sgn)

    maskT = np.ascontiguousarray(mask[0, 0, :128, :128].T, dtype=F32)

    return {"xT": xT, "wq_sb": wq_sb, "wk_sb": wk_sb, "wv_sb": wv_sb,
            "wo_sb": wo_sb,
            "cosm": np.ascontiguousarray(cosT.astype(map_dt)),
            "sinm2": np.ascontiguousarray(sinm2.astype(map_dt)),
            "maskT": maskT, "ident": np.eye(128, dtype=ident_dt)}


def kernel(x, wq, wk, wv, wo, freqs_cos, freqs_sin, mask):
    from concourse import bass_utils

    if "nc" not in _CACHE:
        _CACHE["nc"] = _build_nc(weave=True, tweak=True, den_v=True,
                                 tweak2=True, tweak6=True,
                                 warm2=18, trf16=True, dennorm=False)
    nc = _CACHE["nc"]

    x = np.asarray(x, F32)
    xT_cache = {}
    in_maps = [
        _prep_core_inputs(c, x, np.asarray(wq, F32), np.asarray(wk, F32),
                          np.asarray(wv, F32), np.asarray(wo, F32),
                          np.asarray(freqs_cos, F32), np.asarray(freqs_sin, F32),
                          np.asarray(mask, F32), xT_cache)
        for c in range(NCORES)
    ]
    res = bass_utils.run_bass_kernel_spmd(nc, in_maps, core_ids=list(range(NCORES)))
    out = np.zeros((B, L, D), F32)
    for c in range(NCORES):
        out[c // 4] += res.results[c]["y"]
    return out

